# revision 1
# baseline (speedup 1.0000x reference)
"""Causal self-attention on 8 Trainium2 NeuronCores.

Sharding (matches the batch+head-parallel hint): core c handles batch
b = c // 4 and the 4 heads [hg*4, hg*4+4) where hg = c % 4.  Each core
computes its q/k/v projections from column-sliced c_attn weights, full
causal attention for its heads, and a partial c_proj output from the
matching row slice of w_proj; the host sums the 4 partials per batch.

All matmuls run in float32r (TF32-like rounding, fp32 accumulate).
QKV runs in two token-half stages so the second half's DMA + matmuls
overlap attention on the first half.
"""

import sys

if "/opt/trn_rl_repo" not in sys.path:
    sys.path.insert(0, "/opt/trn_rl_repo")

import numpy as np

import concourse.mybir as mybir
from concourse import bacc
from concourse.bass_utils import run_bass_kernel_spmd
from concourse.tile import TileContext

B, T, C = 2, 2048, 1024
H, D = 16, 64
HL = 4  # heads per core
N_CORES = 8
KT = C // 128  # contraction tiles over the embedding dim
SCALE = 1.0 / 8.0  # 1/sqrt(D)

_CACHE = {}
MM_SITES = {}


def _rec(site, bi):
    MM_SITES[bi.ins.name] = site
    return bi


def _build():
    f32 = mybir.dt.float32
    f32r = mybir.dt.float32r
    nc = bacc.Bacc("TRN2", target_bir_lowering=False, debug=False, num_devices=N_CORES)

    x_in = nc.dram_tensor("x_in", [128, KT, T], f32r, kind="ExternalInput")
    wqk = nc.dram_tensor("wqk", [128, KT, 2 * HL * D], f32r, kind="ExternalInput")
    wv = nc.dram_tensor("wv", [128, KT, HL * D], f32r, kind="ExternalInput")
    wp = nc.dram_tensor("wp", [128, HL // 2, C], f32r, kind="ExternalInput")
    out = nc.dram_tensor("out", [T, C], f32, kind="ExternalOutput")

    with TileContext(nc) as tc:
        with tc.tile_pool(name="persist", bufs=1) as persist:
            # q/k feature-major [d, t]: slot 0/1 = q heads {0,1}/{2,3}, 2/3 = k;
            # split per 512-token block for fine-grained cross-stage deps
            qk_t = [
                [persist.tile([128, 512], f32r, name=f"qk{s}_{tb}") for tb in range(4)]
                for s in range(4)
            ]
            # v token-major per 128-token tile; col D holds ones (denominator)
            v_t = [
                persist.tile([128, HL, D + 1], f32r, name=f"v{tt}") for tt in range(16)
            ]
            # head-pair stacked normalized y per 512-token block
            y2_t = [
                persist.tile([128, HL // 2, 512], f32r, name=f"y2{b_}")
                for b_ in range(4)
            ]
            wp_sb = persist.tile([128, HL // 2, C], f32r)
            nc.sync.dma_start(wp_sb, wp[:, :, :])

            ones32 = persist.tile([128, HL, 1], f32)
            nc.vector.memset(ones32, 1.0)
            # lower-triangular 0/1 mask for the diagonal 128x128 blocks
            tri32 = persist.tile([128, 128], f32)
            nc.vector.memset(tri32, 1.0)
            nc.gpsimd.affine_select(
                out=tri32,
                in_=tri32,
                pattern=[[1, 128]],
                channel_multiplier=-1,
                base=0,
                compare_op=mybir.AluOpType.is_ge,
                fill=0.0,
            )
            tri = persist.tile([128, 128], f32r)
            nc.vector.tensor_copy(tri, tri32)
            # ones row at partition D for the K=1 denominator broadcast
            onesbc32 = persist.tile([D + 1, D], f32)
            nc.vector.memset(onesbc32[D : D + 1, :], 1.0)
            onesbc = persist.tile([D + 1, D], f32r)
            nc.vector.tensor_copy(onesbc[D : D + 1, :], onesbc32[D : D + 1, :])

            with (
                tc.tile_pool(name="qkvp", bufs=1) as qkvp,
                tc.tile_pool(name="attp", bufs=5) as attp,
                tc.tile_pool(name="attsmall", bufs=2) as attsmall,
                tc.tile_pool(name="projp", bufs=2) as projp,
                tc.tile_pool(name="ps_st", bufs=2, space="PSUM") as ps_st,
                tc.tile_pool(name="ps_y", bufs=2, space="PSUM") as ps_y,
                tc.tile_pool(name="ps_share", bufs=2, space="PSUM") as ps_share,
            ):
                # quarter-length x buffers, double-buffered across stages
                x_q = [
                    qkvp.tile([128, KT, T // 4], f32r, name=f"x_q{i}")
                    for i in range(2)
                ]
                wqk_sb = qkvp.tile([128, KT, 2 * HL * D], f32r)
                wv_sb = qkvp.tile([128, KT, HL * D], f32r)
                for jt in range(4):
                    nc.sync.dma_start(
                        wqk_sb[:, :, jt * 128 : (jt + 1) * 128],
                        wqk[:, :, jt * 128 : (jt + 1) * 128],
                    )
                nc.sync.dma_start(wv_sb, wv[:, :, :])

                def qkv_stage(tb):
                    t0 = tb * 512
                    x_sb = x_q[tb % 2]
                    for kt in range(KT):
                        nc.sync.dma_start(x_sb[:, kt, :], x_in[:, kt, t0 : t0 + 512])
                    for jt in range(4):
                        qk_ps = ps_share.tile(
                            [128, 512], f32, tag="share", name="qk_ps"
                        )
                        for kt in range(KT):
                            _rec(
                                "qk",
                                nc.tensor.matmul(
                                    qk_ps,
                                    wqk_sb[:, kt, jt * 128 : (jt + 1) * 128],
                                    x_sb[:, kt, :],
                                    start=(kt == 0),
                                    stop=(kt == KT - 1),
                                ),
                            )
                        nc.vector.tensor_copy(qk_t[jt][tb], qk_ps)
                    for tt2 in range(4):
                        tt = tb * 4 + tt2
                        v_ps = ps_share.tile(
                            [128, HL * D], f32, tag="share", name="v_ps"
                        )
                        for kt in range(KT):
                            _rec(
                                "v",
                                nc.tensor.matmul(
                                    v_ps,
                                    x_sb[:, kt, tt2 * 128 : (tt2 + 1) * 128],
                                    wv_sb[:, kt, :],
                                    start=(kt == 0),
                                    stop=(kt == KT - 1),
                                ),
                            )
                        nc.vector.tensor_copy(
                            v_t[tt][:, :, 0:D],
                            v_ps.rearrange("p (h d) -> p h d", h=HL),
                        )
                        nc.vector.tensor_copy(v_t[tt][:, :, D : D + 1], ones32)

                for tb in range(4):
                    qkv_stage(tb)

                npr = HL // 2

                def proj_block(blk):
                    # c_proj for token block blk, fills attention-phase PE gaps
                    for tt in range(4 * blk, 4 * blk + 4):
                        o_sb = projp.tile([128, C], f32, name="o_sb")
                        off = (tt % 4) * 128
                        for cb in range(2):
                            o_ps = ps_share.tile(
                                [128, 512], f32, tag="share", name="o_ps"
                            )
                            for pr in range(npr):
                                _rec(
                                    "proj",
                                    nc.tensor.matmul(
                                        o_ps,
                                        y2_t[blk][:, pr, off : off + 128],
                                        wp_sb[:, pr, cb * 512 : (cb + 1) * 512],
                                        start=(pr == 0),
                                        stop=(pr == npr - 1),
                                    ),
                                )
                            nc.vector.tensor_copy(
                                o_sb[:, cb * 512 : (cb + 1) * 512], o_ps
                            )
                        nc.sync.dma_start(out[tt * 128 : (tt + 1) * 128, :], o_sb)

                pending_epi = [None]

                def flush_epi():
                    if pending_epi[0] is not None:
                        pending_epi[0]()
                        pending_epi[0] = None

                for jq in range(4):
                    for h in range(HL):
                        qslot = h // 2
                        kslot = 2 + h // 2
                        base = (h % 2) * D
                        pr = h // 2
                        y_ps = ps_y.tile([D + 1, 512], f32, name="y_ps")
                        njt = 4 * (jq + 1)
                        npair = njt // 2

                        def s_pair(p):
                            st = ps_st.tile([128, 2, 512], f32, name="st")
                            est = attp.tile([128, 2, 512], f32r, tag="est", name="est")
                            diag = 2 * p + 1 >= 4 * jq
                            for s in range(2):
                                j = 2 * p + s
                                w = max(0, (j - 4 * jq) * 128)
                                _rec(
                                    "S",
                                    nc.tensor.matmul(
                                        st[:, s, w:],
                                        qk_t[kslot][j // 4][
                                            base : base + D,
                                            (j % 4) * 128 : (j % 4 + 1) * 128,
                                        ],
                                        qk_t[qslot][jq][base : base + D, w:],
                                        start=True,
                                        stop=True,
                                    ),
                                )
                            if not diag:
                                nc.scalar.activation(
                                    est,
                                    st,
                                    mybir.ActivationFunctionType.Exp,
                                    scale=SCALE,
                                )
                            else:
                                for s in range(2):
                                    j = 2 * p + s
                                    w = max(0, (j - 4 * jq) * 128)
                                    nc.scalar.activation(
                                        est[:, s, w:],
                                        st[:, s, w:],
                                        mybir.ActivationFunctionType.Exp,
                                        scale=SCALE,
                                    )
                                    nc.vector.tensor_mul(
                                        est[:, s, w : w + 128],
                                        est[:, s, w : w + 128],
                                        tri,
                                    )
                            return est

                        def pv_pair(p, est):
                            for s in range(2):
                                j = 2 * p + s
                                w = max(0, (j - 4 * jq) * 128)
                                _rec(
                                    "PV",
                                    nc.tensor.matmul(
                                        y_ps[:, w:],
                                        v_t[j][:, h, :],
                                        est[:, s, w:],
                                        start=(j == 0),
                                        stop=(j == njt - 1),
                                    ),
                                )

                        ests = [s_pair(p) for p in range(npair)]
                        for p, est in enumerate(ests):
                            pv_pair(p, est)

                        def epilogue(y_ps=y_ps, jq=jq, h=h, pr=pr):
                            # normalize: row D of y_ps holds the denominator
                            r_sb = attsmall.tile([D + 1, 512], f32r, tag="rr")
                            nc.scalar.copy(r_sb[D : D + 1, :], y_ps[D : D + 1, :])
                            rb_ps = ps_share.tile(
                                [D, 512], f32, tag="share", name="rb_ps"
                            )
                            _rec(
                                "bcast",
                                nc.tensor.matmul(
                                    rb_ps,
                                    onesbc[D : D + 1, :],
                                    r_sb[D : D + 1, :],
                                    start=True,
                                    stop=True,
                                ),
                            )
                            rb_sb = attsmall.tile([D, 512], f32, tag="rb")
                            nc.vector.reciprocal_approx_fast(rb_sb, rb_ps)
                            if h % 2 == 0:
                                nc.vector.tensor_mul(
                                    y2_t[jq][0:D, pr, :], y_ps[0:D, :], rb_sb
                                )
                            else:
                                y_lo = attsmall.tile([D, 512], f32r, tag="ylo")
                                nc.vector.tensor_mul(y_lo, y_ps[0:D, :], rb_sb)
                                nc.gpsimd.dma_start(y2_t[jq][D:128, pr, :], y_lo)

                        flush_epi()
                        pending_epi[0] = epilogue
                    flush_epi()
                    proj_block(jq)

    nc.compile()
    return nc


def _get_nc():
    if "nc" not in _CACHE:
        _CACHE["nc"] = _build()
    return _CACHE["nc"]


def make_in_maps(x, w_attn, w_proj):
    x = np.asarray(x, np.float32)
    w_attn = np.asarray(w_attn, np.float32)
    w_proj = np.asarray(w_proj, np.float32)
    in_maps = []
    for c in range(N_CORES):
        b, hg = c // 4, c % 4
        hs = hg * HL * D  # 256 * hg
        xt = np.ascontiguousarray(x[b].T)  # [C, T]
        x_t = xt.reshape(KT, 128, T).transpose(1, 0, 2)
        wq = w_attn[hs : hs + HL * D, :]
        wk = w_attn[C + hs : C + hs + HL * D, :]
        wqkt = np.concatenate([wq, wk], 0).T  # [C, 512]
        wqk_t = wqkt.reshape(KT, 128, 2 * HL * D).transpose(1, 0, 2)
        wvt = w_attn[2 * C + hs : 2 * C + hs + HL * D, :].T  # [C, 256]
        wv_t = wvt.reshape(KT, 128, HL * D).transpose(1, 0, 2)
        # head-pair stacked rows: [128, HL//2, C]; partition p of pair pr is
        # local feature pr*128 + p (head 2*pr dims then head 2*pr+1 dims)
        wp_t = (
            w_proj[:, hs : hs + HL * D].T.reshape(HL // 2, 128, C).transpose(1, 0, 2)
        )
        in_maps.append(
            {
                "x_in": np.ascontiguousarray(x_t, np.float32),
                "wqk": np.ascontiguousarray(wqk_t, np.float32),
                "wv": np.ascontiguousarray(wv_t, np.float32),
                "wp": np.ascontiguousarray(wp_t, np.float32),
            }
        )
    return in_maps


def run(in_maps, **kwargs):
    nc = _get_nc()
    return run_bass_kernel_spmd(nc, in_maps, core_ids=list(range(N_CORES)), **kwargs)


def combine(results):
    out = np.zeros((B, T, C), np.float64)
    for c in range(N_CORES):
        out[c // 4] += results[c]["out"].astype(np.float64)
    return out.astype(np.float32)


def kernel(x, w_attn, w_proj):
    res = run(make_in_maps(x, w_attn, w_proj))
    return combine(res.results)



# revision 5
# speedup vs baseline: 1.3221x; 1.3221x over previous
"""Causal self-attention on 8 Trainium2 NeuronCores.

Sharding (batch + head-parallel): core c handles batch b = c // 4 and the
4 heads [hg*4, hg*4+4) where hg = c % 4.  Each core computes q/k/v from
column-sliced c_attn weights, full causal attention for its heads, and a
partial c_proj output from the matching row slice of w_proj; the host
sums the 4 partials per batch.

All matmul inputs are bf16 (fp32 PSUM accumulate).  The schedule keeps
the PE array continuously busy so the HAM clock gate stays at full rate:
 - inputs stream in bf16 with the first token block prioritized,
 - dummy warm-up matmuls run while the first DMAs land,
 - in the attention phase, S matmuls of head h interleave with PV
   matmuls of head h-1, with next-block QKV and previous-block c_proj
   matmuls sprinkled in as fillers, so exp latency (scalar engine)
   never stalls the PE.
"""

import sys

if "/opt/trn_rl_repo" not in sys.path:
    sys.path.insert(0, "/opt/trn_rl_repo")

from collections import deque

import ml_dtypes
import numpy as np

import concourse.mybir as mybir
from concourse import bacc
from concourse.bass_utils import run_bass_kernel_spmd
from concourse.tile import TileContext

B, T, C = 2, 2048, 1024
H, D = 16, 64
HL = 4  # heads per core
N_CORES = 8
KT = C // 128  # contraction tiles over the embedding dim
SCALE = 1.0 / 8.0  # 1/sqrt(D)

_CACHE = {}


def _build():
    f32 = mybir.dt.float32
    f32r = mybir.dt.float32r
    bf16 = mybir.dt.bfloat16
    EXP = mybir.ActivationFunctionType.Exp
    nc = bacc.Bacc("TRN2", target_bir_lowering=False, debug=False, num_devices=N_CORES)

    x_in = nc.dram_tensor("x_in", [128, KT, T], bf16, kind="ExternalInput")
    wqk = nc.dram_tensor("wqk", [128, KT, 2 * HL * D], bf16, kind="ExternalInput")
    wv = nc.dram_tensor("wv", [128, KT, HL * D], bf16, kind="ExternalInput")
    wp = nc.dram_tensor("wp", [128, HL // 2, C], bf16, kind="ExternalInput")
    out = nc.dram_tensor("out", [T, C], f32, kind="ExternalOutput")

    with TileContext(nc) as tc:
        with tc.tile_pool(name="persist", bufs=1) as persist:
            x_sb = persist.tile([128, KT, T], bf16)
            wqk_sb = persist.tile([128, KT, 2 * HL * D], bf16)
            wv_sb = persist.tile([128, KT, HL * D], bf16)
            wp_sb = persist.tile([128, HL // 2, C], bf16)
            # q/k feature-major [d, t]: slot 0/1 = q heads {0,1}/{2,3}, 2/3 = k
            qk_t = [
                [persist.tile([128, 512], bf16, name=f"qk{s}_{tb}") for tb in range(4)]
                for s in range(4)
            ]
            # v token-major per 128-token tile; col D holds ones (denominator)
            v_t = [
                persist.tile([128, HL, D + 1], bf16, name=f"v{tt}") for tt in range(16)
            ]
            # head-pair stacked normalized y per 512-token block
            y2_t = [
                persist.tile([128, HL // 2, 512], bf16, name=f"y2{b_}")
                for b_ in range(4)
            ]
            warm = persist.tile([128, 512], bf16)

            # input DMAs in priority order: first token block + qkv weights
            # first, then the rest of x, then the (last-used) c_proj weights
            nc.sync.dma_start(x_sb[:, :, 0:512], x_in[:, :, 0:512])
            nc.sync.dma_start(wqk_sb, wqk[:, :, :])
            nc.sync.dma_start(wv_sb, wv[:, :, :])
            for tb in range(1, 4):
                nc.sync.dma_start(
                    x_sb[:, :, tb * 512 : (tb + 1) * 512],
                    x_in[:, :, tb * 512 : (tb + 1) * 512],
                )
            nc.sync.dma_start(wp_sb, wp[:, :, :])

            # constants
            nc.vector.memset(warm, 0.125)
            ones_b = persist.tile([128, HL, 1], bf16)
            nc.vector.memset(ones_b, 1.0)
            for tt in range(16):
                nc.vector.tensor_copy(v_t[tt][:, :, D : D + 1], ones_b)
            # lower-triangular 0/1 mask for the diagonal 128x128 blocks
            tri32 = persist.tile([128, 128], f32)
            nc.vector.memset(tri32, 1.0)
            nc.gpsimd.affine_select(
                out=tri32,
                in_=tri32,
                pattern=[[1, 128]],
                channel_multiplier=-1,
                base=0,
                compare_op=mybir.AluOpType.is_ge,
                fill=0.0,
            )
            tri = persist.tile([128, 128], bf16)
            nc.vector.tensor_copy(tri, tri32)
            # ones row at partition D for the K=1 denominator broadcast
            onesbc32 = persist.tile([D + 1, D], f32)
            nc.vector.memset(onesbc32[D : D + 1, :], 1.0)
            onesbc = persist.tile([D + 1, D], f32r)
            nc.vector.tensor_copy(onesbc[D : D + 1, :], onesbc32[D : D + 1, :])

            with (
                tc.tile_pool(name="attp", bufs=14) as attp,
                tc.tile_pool(name="attsmall", bufs=2) as attsmall,
                tc.tile_pool(name="projp", bufs=2) as projp,
                tc.tile_pool(name="ps_st", bufs=2, space="PSUM") as ps_st,
                tc.tile_pool(name="ps_y", bufs=2, space="PSUM") as ps_y,
                tc.tile_pool(name="ps_share", bufs=2, space="PSUM") as ps_share,
            ):
                # dummy matmuls: keep the PE busy while the first input DMAs
                # land so the HAM clock gate releases before real work starts
                for i in range(8):
                    wps = ps_share.tile([128, 512], f32, tag="share", name="warm_ps")
                    nc.tensor.matmul(wps, warm[:, 0:128], warm, start=True, stop=True)

                def qk_chunk(tb, jt):
                    def go():
                        qk_ps = ps_share.tile(
                            [128, 512], f32, tag="share", name="qk_ps"
                        )
                        for kt in range(KT):
                            nc.tensor.matmul(
                                qk_ps,
                                wqk_sb[:, kt, jt * 128 : (jt + 1) * 128],
                                x_sb[:, kt, tb * 512 : (tb + 1) * 512],
                                start=(kt == 0),
                                stop=(kt == KT - 1),
                            )
                        nc.vector.tensor_copy(qk_t[jt][tb], qk_ps)

                    return go

                def v_chunk(tb, tt2):
                    def go():
                        tt = tb * 4 + tt2
                        v_ps = ps_share.tile(
                            [128, HL * D], f32, tag="share", name="v_ps"
                        )
                        for kt in range(KT):
                            nc.tensor.matmul(
                                v_ps,
                                x_sb[:, kt, tt * 128 : (tt + 1) * 128],
                                wv_sb[:, kt, :],
                                start=(kt == 0),
                                stop=(kt == KT - 1),
                            )
                        nc.vector.tensor_copy(
                            v_t[tt][:, :, 0:D],
                            v_ps.rearrange("p (h d) -> p h d", h=HL),
                        )

                    return go

                def proj_chunk(blk, tt):
                    def go():
                        o_sb = projp.tile([128, C], f32, name="o_sb")
                        off = (tt % 4) * 128
                        for cb in range(2):
                            o_ps = ps_share.tile(
                                [128, 512], f32, tag="share", name="o_ps"
                            )
                            for pr in range(2):
                                nc.tensor.matmul(
                                    o_ps,
                                    y2_t[blk][:, pr, off : off + 128],
                                    wp_sb[:, pr, cb * 512 : (cb + 1) * 512],
                                    start=(pr == 0),
                                    stop=(pr == 1),
                                )
                            nc.vector.tensor_copy(
                                o_sb[:, cb * 512 : (cb + 1) * 512], o_ps
                            )
                        nc.gpsimd.dma_start(out[tt * 128 : (tt + 1) * 128, :], o_sb)

                    return go

                # qkv for token block 0 up front (q/k for heads 0/1 first so
                # attention can begin as early as possible)
                for jt in (0, 2):
                    qk_chunk(0, jt)()
                for tt2 in range(4):
                    v_chunk(0, tt2)()
                for jt in (1, 3):
                    qk_chunk(0, jt)()

                def s_pair(jq, h, p, ests):
                    qslot = h // 2
                    kslot = 2 + h // 2
                    base = (h % 2) * D
                    st = ps_st.tile([128, 2, 512], f32, name="st")
                    est = attp.tile([128, 2, 512], bf16, tag="est", name="est")
                    diag = 2 * p + 1 >= 4 * jq
                    for s in range(2):
                        j = 2 * p + s
                        w = max(0, (j - 4 * jq) * 128)
                        nc.tensor.matmul(
                            st[:, s, w:],
                            qk_t[kslot][j // 4][
                                base : base + D,
                                (j % 4) * 128 : (j % 4 + 1) * 128,
                            ],
                            qk_t[qslot][jq][base : base + D, w:],
                            start=True,
                            stop=True,
                        )
                    if not diag:
                        nc.scalar.activation(est, st, EXP, scale=SCALE)
                    else:
                        for s in range(2):
                            j = 2 * p + s
                            w = max(0, (j - 4 * jq) * 128)
                            nc.scalar.activation(
                                est[:, s, w:], st[:, s, w:], EXP, scale=SCALE
                            )
                            nc.vector.tensor_mul(
                                est[:, s, w : w + 128], est[:, s, w : w + 128], tri
                            )
                    ests.append(est)

                class Pending:
                    def __init__(self, jq, h, ests):
                        self.jq, self.h, self.ests = jq, h, ests
                        self.njt = 4 * (jq + 1)
                        self.p = 0
                        self.y_ps = ps_y.tile([D + 1, 512], f32, name="y_ps")

                    def step(self):
                        if self.p >= len(self.ests):
                            return False
                        est = self.ests[self.p]
                        for s in range(2):
                            j = 2 * self.p + s
                            w = max(0, (j - 4 * self.jq) * 128)
                            nc.tensor.matmul(
                                self.y_ps[:, w:],
                                v_t[j][:, self.h, :],
                                est[:, s, w:],
                                start=(j == 0),
                                stop=(j == self.njt - 1),
                            )
                        self.p += 1
                        return True

                    def finish(self):
                        while self.step():
                            pass
                        jq, h, y_ps = self.jq, self.h, self.y_ps
                        pr = h // 2
                        # normalize: row D of y_ps holds the denominator
                        r_sb = attsmall.tile([D + 1, 512], f32r, tag="rr")
                        nc.vector.tensor_copy(r_sb[D : D + 1, :], y_ps[D : D + 1, :])
                        rb_ps = ps_share.tile(
                            [D, 512], f32, tag="share", name="rb_ps"
                        )
                        nc.tensor.matmul(
                            rb_ps,
                            onesbc[D : D + 1, :],
                            r_sb[D : D + 1, :],
                            start=True,
                            stop=True,
                        )
                        rb_sb = attsmall.tile([D, 512], f32, tag="rb")
                        nc.vector.reciprocal_approx_fast(rb_sb, rb_ps)
                        if h % 2 == 0:
                            nc.vector.tensor_mul(
                                y2_t[jq][0:D, pr, :], y_ps[0:D, :], rb_sb
                            )
                        else:
                            y_lo = attsmall.tile([D, 512], bf16, tag="ylo")
                            nc.vector.tensor_mul(y_lo, y_ps[0:D, :], rb_sb)
                            nc.gpsimd.dma_start(y2_t[jq][D:128, pr, :], y_lo)

                pend = [None]

                def drain_pend():
                    if pend[0] is not None:
                        pend[0].finish()
                        pend[0] = None

                fillers = deque()
                for jq in range(4):
                    fillers.clear()
                    if jq < 3:
                        for jt in range(4):
                            fillers.append(("qkv", qk_chunk(jq + 1, jt)))
                        for tt2 in range(4):
                            fillers.append(("qkv", v_chunk(jq + 1, tt2)))
                    if jq > 0:
                        for tt2 in range(4):
                            # proj of block jq-1 reads y2_t[jq-1], whose last
                            # slice is written by the epilogue of (jq-1, h3) --
                            # issued at the drain at the end of h0's pair loop,
                            # so proj fillers may only pop from h >= 1
                            fillers.append(
                                ("proj", proj_chunk(jq - 1, (jq - 1) * 4 + tt2))
                            )
                    npair = 2 * (jq + 1)
                    nslots = HL * npair
                    fcount = len(fillers)
                    slot = 0
                    fdone = 0
                    for h in range(HL):
                        ests = []
                        for p in range(npair):
                            s_pair(jq, h, p, ests)
                            if pend[0] is not None:
                                pend[0].step()
                            slot += 1
                            want = fcount * slot // nslots
                            while (
                                fdone < want
                                and fillers
                                and (h >= 1 or fillers[0][0] == "qkv")
                            ):
                                fillers.popleft()[1]()
                                fdone += 1
                        drain_pend()
                        pend[0] = Pending(jq, h, ests)
                    if jq < 3:
                        while fillers:
                            fillers.popleft()[1]()

                # tail: PV of the last head interleaved with proj of block 2,
                # then the epilogue and proj of block 3
                while (pend[0] is not None and pend[0].p < len(pend[0].ests)) or fillers:
                    if pend[0] is not None:
                        pend[0].step()
                    if fillers:
                        fillers.popleft()[1]()
                drain_pend()
                for tt2 in range(4):
                    proj_chunk(3, 12 + tt2)()

    nc.compile()
    return nc


def _get_nc():
    if "nc" not in _CACHE:
        _CACHE["nc"] = _build()
    return _CACHE["nc"]


def make_in_maps(x, w_attn, w_proj):
    x = np.asarray(x, np.float32)
    w_attn = np.asarray(w_attn, np.float32)
    w_proj = np.asarray(w_proj, np.float32)
    bf16 = ml_dtypes.bfloat16
    in_maps = []
    for c in range(N_CORES):
        b, hg = c // 4, c % 4
        hs = hg * HL * D  # 256 * hg
        xt = np.ascontiguousarray(x[b].T)  # [C, T]
        x_t = xt.reshape(KT, 128, T).transpose(1, 0, 2)
        wq = w_attn[hs : hs + HL * D, :]
        wk = w_attn[C + hs : C + hs + HL * D, :]
        wqkt = np.concatenate([wq, wk], 0).T  # [C, 512]
        wqk_t = wqkt.reshape(KT, 128, 2 * HL * D).transpose(1, 0, 2)
        wvt = w_attn[2 * C + hs : 2 * C + hs + HL * D, :].T  # [C, 256]
        wv_t = wvt.reshape(KT, 128, HL * D).transpose(1, 0, 2)
        # head-pair stacked rows: [128, HL//2, C]; partition p of pair pr is
        # local feature pr*128 + p (head 2*pr dims then head 2*pr+1 dims)
        wp_t = (
            w_proj[:, hs : hs + HL * D].T.reshape(HL // 2, 128, C).transpose(1, 0, 2)
        )
        in_maps.append(
            {
                "x_in": np.ascontiguousarray(x_t).astype(bf16),
                "wqk": np.ascontiguousarray(wqk_t).astype(bf16),
                "wv": np.ascontiguousarray(wv_t).astype(bf16),
                "wp": np.ascontiguousarray(wp_t).astype(bf16),
            }
        )
    return in_maps


def run(in_maps, **kwargs):
    nc = _get_nc()
    return run_bass_kernel_spmd(nc, in_maps, core_ids=list(range(N_CORES)), **kwargs)


def combine(results):
    out = np.zeros((B, T, C), np.float64)
    for c in range(N_CORES):
        out[c // 4] += results[c]["out"].astype(np.float64)
    return out.astype(np.float32)


def kernel(x, w_attn, w_proj):
    res = run(make_in_maps(x, w_attn, w_proj))
    return combine(res.results)


# revision 11
# speedup vs baseline: 1.3519x; 1.0225x over previous
"""Causal self-attention on 8 Trainium2 NeuronCores.

Sharding (batch + head-parallel): core c handles batch b = c // 4 and the
4 heads [hg*4, hg*4+4) where hg = c % 4.  Each core computes q/k/v from
column-sliced c_attn weights, full causal attention for its heads, and a
partial c_proj output from the matching row slice of w_proj; the host
sums the 4 partials per batch.

All matmul inputs are bf16 (fp32 PSUM accumulate).  The schedule keeps
the PE array continuously busy so the HAM clock gate stays at full rate:
 - inputs stream in bf16 with the first token block prioritized,
 - dummy warm-up matmuls run while the first DMAs land,
 - in the attention phase, S matmuls of head h interleave with PV
   matmuls of head h-1, with next-block QKV and previous-block c_proj
   matmuls sprinkled in as fillers, so exp latency (scalar engine)
   never stalls the PE.
"""

import sys

if "/opt/trn_rl_repo" not in sys.path:
    sys.path.insert(0, "/opt/trn_rl_repo")

from collections import deque

import ml_dtypes
import numpy as np

import concourse.mybir as mybir
from concourse import bacc
from concourse.bass_utils import run_bass_kernel_spmd
from concourse.tile import TileContext

B, T, C = 2, 2048, 1024
H, D = 16, 64
HL = 4  # heads per core
N_CORES = 8
KT = C // 128  # contraction tiles over the embedding dim
SCALE = 1.0 / 8.0  # 1/sqrt(D)

_CACHE = {}


def _build():
    f32 = mybir.dt.float32
    f32r = mybir.dt.float32r
    bf16 = mybir.dt.bfloat16
    EXP = mybir.ActivationFunctionType.Exp
    nc = bacc.Bacc("TRN2", target_bir_lowering=False, debug=False, num_devices=N_CORES)

    x_in = nc.dram_tensor("x_in", [128, KT, T], bf16, kind="ExternalInput")
    wqk = nc.dram_tensor("wqk", [128, KT, 2 * HL * D], bf16, kind="ExternalInput")
    wv = nc.dram_tensor("wv", [128, KT, HL * D], bf16, kind="ExternalInput")
    wp = nc.dram_tensor("wp", [128, HL // 2, C], bf16, kind="ExternalInput")
    out = nc.dram_tensor("out", [T, C], bf16, kind="ExternalOutput")
    # pr1 half of token block 3 lands separately so its c_proj matmuls can
    # start before the last head's epilogue; the host sums the two halves
    out_b = nc.dram_tensor("out_b", [512, C], bf16, kind="ExternalOutput")

    with TileContext(nc) as tc:
        with tc.tile_pool(name="persist", bufs=1) as persist:
            x_sb = persist.tile([128, KT, T], bf16)
            wqk_sb = persist.tile([128, KT, 2 * HL * D], bf16)
            wv_sb = persist.tile([128, KT, HL * D], bf16)
            wp_sb = persist.tile([128, HL // 2, C], bf16)
            # q/k feature-major [d, t]: slot 0/1 = q heads {0,1}/{2,3}, 2/3 = k
            qk_t = [
                [persist.tile([128, 512], bf16, name=f"qk{s}_{tb}") for tb in range(4)]
                for s in range(4)
            ]
            # v token-major per 128-token tile; col D holds ones (denominator)
            v_t = [
                persist.tile([128, HL, D + 1], bf16, name=f"v{tt}") for tt in range(16)
            ]
            # head-pair stacked normalized y per 512-token block
            y2_t = [
                persist.tile([128, HL // 2, 512], bf16, name=f"y2{b_}")
                for b_ in range(4)
            ]
            warm = persist.tile([128, 512], bf16)

            # input DMAs in priority order: q/k weights for heads 0/1, the
            # first token block of x, then the rest in first-use order
            nc.sync.dma_start(wqk_sb[:, :, 0:128], wqk[:, :, 0:128])
            nc.sync.dma_start(wqk_sb[:, :, 256:384], wqk[:, :, 256:384])
            nc.sync.dma_start(x_sb[:, :, 0:512], x_in[:, :, 0:512])
            nc.sync.dma_start(wv_sb, wv[:, :, :])
            nc.sync.dma_start(wqk_sb[:, :, 128:256], wqk[:, :, 128:256])
            nc.sync.dma_start(wqk_sb[:, :, 384:512], wqk[:, :, 384:512])
            for tb in range(1, 4):
                nc.sync.dma_start(
                    x_sb[:, :, tb * 512 : (tb + 1) * 512],
                    x_in[:, :, tb * 512 : (tb + 1) * 512],
                )
            nc.sync.dma_start(wp_sb, wp[:, :, :])

            # constants
            nc.vector.memset(warm, 0.125)
            ones_b = persist.tile([128, HL, 1], bf16)
            nc.vector.memset(ones_b, 1.0)
            for tt in range(16):
                nc.vector.tensor_copy(v_t[tt][:, :, D : D + 1], ones_b)
            # lower-triangular 0/1 mask for the diagonal 128x128 blocks
            tri32 = persist.tile([128, 128], f32)
            nc.vector.memset(tri32, 1.0)
            nc.gpsimd.affine_select(
                out=tri32,
                in_=tri32,
                pattern=[[1, 128]],
                channel_multiplier=-1,
                base=0,
                compare_op=mybir.AluOpType.is_ge,
                fill=0.0,
            )
            tri = persist.tile([128, 128], bf16)
            nc.vector.tensor_copy(tri, tri32)
            # ones row at partition D for the K=1 denominator broadcast
            onesbc32 = persist.tile([D + 1, D], f32)
            nc.vector.memset(onesbc32[D : D + 1, :], 1.0)
            onesbc = persist.tile([D + 1, D], f32r)
            nc.vector.tensor_copy(onesbc[D : D + 1, :], onesbc32[D : D + 1, :])

            with (
                tc.tile_pool(name="attp", bufs=14) as attp,
                tc.tile_pool(name="attsmall", bufs=2) as attsmall,
                tc.tile_pool(name="projp", bufs=2) as projp,
                tc.tile_pool(name="ps_st", bufs=2, space="PSUM") as ps_st,
                tc.tile_pool(name="ps_y", bufs=2, space="PSUM") as ps_y,
                tc.tile_pool(name="ps_share", bufs=2, space="PSUM") as ps_share,
            ):
                # dummy matmuls: keep the PE busy while the first input DMAs
                # land so the HAM clock gate releases before real work starts
                for i in range(4):
                    wps = ps_share.tile([128, 512], f32, tag="share", name="warm_ps")
                    nc.tensor.matmul(wps, warm[:, 0:128], warm, start=True, stop=True)

                def qk_chunk(tb, jt):
                    def go():
                        qk_ps = ps_share.tile(
                            [128, 512], f32, tag="share", name="qk_ps"
                        )
                        for kt in range(KT):
                            nc.tensor.matmul(
                                qk_ps,
                                wqk_sb[:, kt, jt * 128 : (jt + 1) * 128],
                                x_sb[:, kt, tb * 512 : (tb + 1) * 512],
                                start=(kt == 0),
                                stop=(kt == KT - 1),
                            )
                        nc.vector.tensor_copy(qk_t[jt][tb], qk_ps)

                    return go

                def v_chunk(tb, tt2):
                    def go():
                        tt = tb * 4 + tt2
                        v_ps = ps_share.tile(
                            [128, HL * D], f32, tag="share", name="v_ps"
                        )
                        for kt in range(KT):
                            nc.tensor.matmul(
                                v_ps,
                                x_sb[:, kt, tt * 128 : (tt + 1) * 128],
                                wv_sb[:, kt, :],
                                start=(kt == 0),
                                stop=(kt == KT - 1),
                            )
                        nc.vector.tensor_copy(
                            v_t[tt][:, :, 0:D],
                            v_ps.rearrange("p (h d) -> p h d", h=HL),
                        )

                    return go

                def proj_chunk(blk, tt, prs=(0, 1), dst=None):
                    def go():
                        o_sb = projp.tile([128, C], bf16, name="o_sb")
                        off = (tt % 4) * 128
                        for cb in range(2):
                            o_ps = ps_share.tile(
                                [128, 512], f32, tag="share", name="o_ps"
                            )
                            for i, pr in enumerate(prs):
                                nc.tensor.matmul(
                                    o_ps,
                                    y2_t[blk][:, pr, off : off + 128],
                                    wp_sb[:, pr, cb * 512 : (cb + 1) * 512],
                                    start=(i == 0),
                                    stop=(i == len(prs) - 1),
                                )
                            nc.vector.tensor_copy(
                                o_sb[:, cb * 512 : (cb + 1) * 512], o_ps
                            )
                        d = out[tt * 128 : (tt + 1) * 128, :] if dst is None else dst
                        nc.gpsimd.dma_start(d, o_sb)

                    return go

                # q/k for heads 0/1 of token block 0 up front; the rest of
                # block 0's qkv runs as early fillers inside the jq0 loop
                for jt in (0, 2):
                    qk_chunk(0, jt)()

                def s_pair(jq, h, p, ests):
                    qslot = h // 2
                    kslot = 2 + h // 2
                    base = (h % 2) * D
                    st = ps_st.tile([128, 2, 512], f32, name="st")
                    est = attp.tile([128, 2, 512], bf16, tag="est", name="est")
                    diag = 2 * p + 1 >= 4 * jq
                    for s in range(2):
                        j = 2 * p + s
                        w = max(0, (j - 4 * jq) * 128)
                        nc.tensor.matmul(
                            st[:, s, w:],
                            qk_t[kslot][j // 4][
                                base : base + D,
                                (j % 4) * 128 : (j % 4 + 1) * 128,
                            ],
                            qk_t[qslot][jq][base : base + D, w:],
                            start=True,
                            stop=True,
                        )
                    if not diag:
                        nc.scalar.activation(est, st, EXP, scale=SCALE)
                    else:
                        for s in range(2):
                            j = 2 * p + s
                            w = max(0, (j - 4 * jq) * 128)
                            nc.scalar.activation(
                                est[:, s, w:], st[:, s, w:], EXP, scale=SCALE
                            )
                            nc.vector.tensor_mul(
                                est[:, s, w : w + 128], est[:, s, w : w + 128], tri
                            )
                    ests.append(est)

                class Pending:
                    def __init__(self, jq, h, ests):
                        self.jq, self.h, self.ests = jq, h, ests
                        self.njt = 4 * (jq + 1)
                        self.p = 0
                        self.y_ps = ps_y.tile([D + 1, 512], f32, name="y_ps")

                    def step(self):
                        if self.p >= len(self.ests):
                            return False
                        est = self.ests[self.p]
                        for s in range(2):
                            j = 2 * self.p + s
                            w = max(0, (j - 4 * self.jq) * 128)
                            nc.tensor.matmul(
                                self.y_ps[:, w:],
                                v_t[j][:, self.h, :],
                                est[:, s, w:],
                                start=(j == 0),
                                stop=(j == self.njt - 1),
                            )
                        self.p += 1
                        return True

                    def finish(self):
                        while self.step():
                            pass
                        jq, h, y_ps = self.jq, self.h, self.y_ps
                        pr = h // 2
                        # normalize: row D of y_ps holds the denominator
                        r_sb = attsmall.tile([D + 1, 512], f32r, tag="rr")
                        nc.vector.tensor_copy(r_sb[D : D + 1, :], y_ps[D : D + 1, :])
                        rb_ps = ps_share.tile(
                            [D, 512], f32, tag="share", name="rb_ps"
                        )
                        nc.tensor.matmul(
                            rb_ps,
                            onesbc[D : D + 1, :],
                            r_sb[D : D + 1, :],
                            start=True,
                            stop=True,
                        )
                        rb_sb = attsmall.tile([D, 512], f32, tag="rb")
                        nc.vector.reciprocal_approx_fast(rb_sb, rb_ps)
                        if h % 2 == 0:
                            nc.vector.tensor_mul(
                                y2_t[jq][0:D, pr, :], y_ps[0:D, :], rb_sb
                            )
                        else:
                            y_lo = attsmall.tile([D, 512], bf16, tag="ylo")
                            nc.vector.tensor_mul(y_lo, y_ps[0:D, :], rb_sb)
                            nc.gpsimd.dma_start(y2_t[jq][D:128, pr, :], y_lo)

                pend = [None]

                def drain_pend():
                    if pend[0] is not None:
                        pend[0].finish()
                        pend[0] = None

                # filler gating: "qkv" chunks are safe anywhere; "proj" reads
                # y2_t[jq-1] whose last slice is written by the epilogue of
                # (jq-1, h3), issued at the drain ending h0's pair loop, so it
                # may only pop from h >= 1; "proj3a" reads y2_t[3] pr0 written
                # by the epilogues of (jq3, h0/h1), so it may only pop in h3
                min_h = {"qkv": 0, "proj": 1, "proj3a": 3}
                fillers = deque()
                for jq in range(4):
                    fillers.clear()
                    if jq == 0:
                        for tt2 in range(4):
                            fillers.append(("qkv", v_chunk(0, tt2)))
                        for jt in (1, 3):
                            fillers.append(("qkv", qk_chunk(0, jt)))
                    if jq < 3:
                        for jt in range(4):
                            fillers.append(("qkv", qk_chunk(jq + 1, jt)))
                        for tt2 in range(4):
                            fillers.append(("qkv", v_chunk(jq + 1, tt2)))
                    if jq > 0:
                        for tt2 in range(4):
                            fillers.append(
                                ("proj", proj_chunk(jq - 1, (jq - 1) * 4 + tt2))
                            )
                    if jq == 3:
                        for tt2 in range(4):
                            fillers.append(
                                ("proj3a", proj_chunk(3, 12 + tt2, prs=(0,)))
                            )
                    npair = 2 * (jq + 1)
                    nslots = HL * npair
                    fcount = len(fillers)
                    slot = 0
                    fdone = 0
                    for h in range(HL):
                        ests = []
                        for p in range(npair):
                            s_pair(jq, h, p, ests)
                            slot += 1
                            want = fcount * slot // nslots
                            while (
                                fdone < want
                                and fillers
                                and h >= min_h[fillers[0][0]]
                            ):
                                fillers.popleft()[1]()
                                fdone += 1
                            if pend[0] is not None:
                                pend[0].step()
                        drain_pend()
                        pend[0] = Pending(jq, h, ests)
                    if jq < 3:
                        while fillers:
                            fillers.popleft()[1]()

                # tail: PV of the last head interleaved with leftover fillers,
                # then the epilogue and the pr1 half of block 3's c_proj
                while (pend[0] is not None and pend[0].p < len(pend[0].ests)) or fillers:
                    if pend[0] is not None:
                        pend[0].step()
                    if fillers:
                        fillers.popleft()[1]()
                drain_pend()
                for tt2 in range(4):
                    proj_chunk(
                        3,
                        12 + tt2,
                        prs=(1,),
                        dst=out_b[tt2 * 128 : (tt2 + 1) * 128, :],
                    )()

    nc.compile()
    return nc


def _get_nc():
    if "nc" not in _CACHE:
        _CACHE["nc"] = _build()
    return _CACHE["nc"]


def make_in_maps(x, w_attn, w_proj):
    x = np.asarray(x, np.float32)
    w_attn = np.asarray(w_attn, np.float32)
    w_proj = np.asarray(w_proj, np.float32)
    bf16 = ml_dtypes.bfloat16
    in_maps = []
    for c in range(N_CORES):
        b, hg = c // 4, c % 4
        hs = hg * HL * D  # 256 * hg
        xt = np.ascontiguousarray(x[b].T)  # [C, T]
        x_t = xt.reshape(KT, 128, T).transpose(1, 0, 2)
        wq = w_attn[hs : hs + HL * D, :]
        wk = w_attn[C + hs : C + hs + HL * D, :]
        wqkt = np.concatenate([wq, wk], 0).T  # [C, 512]
        wqk_t = wqkt.reshape(KT, 128, 2 * HL * D).transpose(1, 0, 2)
        wvt = w_attn[2 * C + hs : 2 * C + hs + HL * D, :].T  # [C, 256]
        wv_t = wvt.reshape(KT, 128, HL * D).transpose(1, 0, 2)
        # head-pair stacked rows: [128, HL//2, C]; partition p of pair pr is
        # local feature pr*128 + p (head 2*pr dims then head 2*pr+1 dims)
        wp_t = (
            w_proj[:, hs : hs + HL * D].T.reshape(HL // 2, 128, C).transpose(1, 0, 2)
        )
        in_maps.append(
            {
                "x_in": np.ascontiguousarray(x_t).astype(bf16),
                "wqk": np.ascontiguousarray(wqk_t).astype(bf16),
                "wv": np.ascontiguousarray(wv_t).astype(bf16),
                "wp": np.ascontiguousarray(wp_t).astype(bf16),
            }
        )
    return in_maps


def run(in_maps, **kwargs):
    nc = _get_nc()
    return run_bass_kernel_spmd(nc, in_maps, core_ids=list(range(N_CORES)), **kwargs)


def combine(results):
    out = np.zeros((B, T, C), np.float64)
    for c in range(N_CORES):
        out[c // 4] += results[c]["out"].astype(np.float64)
        # token block 3 was written pr-split: "out" rows 1536: hold the pr0
        # half, "out_b" the pr1 half
        out[c // 4][3 * 512 :] += results[c]["out_b"].astype(np.float64)
    return out.astype(np.float32)


def kernel(x, w_attn, w_proj):
    res = run(make_in_maps(x, w_attn, w_proj))
    return combine(res.results)


# revision 17
# speedup vs baseline: 1.3669x; 1.0111x over previous
"""Causal self-attention on 8 Trainium2 NeuronCores.

Sharding (batch + head-parallel): core c handles batch b = c // 4 and the
4 heads [hg*4, hg*4+4) where hg = c % 4.  Each core computes q/k/v from
column-sliced c_attn weights, full causal attention for its heads, and a
partial c_proj output from the matching row slice of w_proj; the host
sums the 4 partials per batch.

All matmul inputs are bf16 (fp32 PSUM accumulate).  The schedule keeps
the PE array continuously busy so the HAM clock gate stays at full rate:
 - inputs stream in bf16 with the first token block prioritized,
 - dummy warm-up matmuls run while the first DMAs land,
 - in the attention phase, S matmuls of head h interleave with PV
   matmuls of head h-1, with next-block QKV and previous-block c_proj
   matmuls sprinkled in as fillers, so exp latency (scalar engine)
   never stalls the PE.
"""

import sys

if "/opt/trn_rl_repo" not in sys.path:
    sys.path.insert(0, "/opt/trn_rl_repo")

from collections import deque

import ml_dtypes
import numpy as np

import concourse.mybir as mybir
from concourse import bacc
from concourse.bass_utils import run_bass_kernel_spmd
from concourse.tile import TileContext

B, T, C = 2, 2048, 1024
H, D = 16, 64
HL = 4  # heads per core
N_CORES = 8
KT = C // 128  # contraction tiles over the embedding dim
SCALE = 1.0 / 8.0  # 1/sqrt(D)

_CACHE = {}


def _build():
    f32 = mybir.dt.float32
    f32r = mybir.dt.float32r
    bf16 = mybir.dt.bfloat16
    EXP = mybir.ActivationFunctionType.Exp
    nc = bacc.Bacc("TRN2", target_bir_lowering=False, debug=False, num_devices=N_CORES)

    x_in = nc.dram_tensor("x_in", [128, KT, T], bf16, kind="ExternalInput")
    wqk = nc.dram_tensor("wqk", [128, KT, 2 * HL * D], bf16, kind="ExternalInput")
    wv = nc.dram_tensor("wv", [128, KT, HL * D], bf16, kind="ExternalInput")
    wp = nc.dram_tensor("wp", [128, HL // 2, C], bf16, kind="ExternalInput")
    out = nc.dram_tensor("out", [T, C], bf16, kind="ExternalOutput")
    # pr1 half of token block 3 lands separately so its c_proj matmuls can
    # start before the last head's epilogue; the host sums the two halves
    out_b = nc.dram_tensor("out_b", [512, C], bf16, kind="ExternalOutput")

    with TileContext(nc) as tc:
        with tc.tile_pool(name="persist", bufs=1) as persist:
            x_sb = persist.tile([128, KT, T], bf16)
            wqk_sb = persist.tile([128, KT, 2 * HL * D], bf16)
            wv_sb = persist.tile([128, KT, HL * D], bf16)
            wp_sb = persist.tile([128, HL // 2, C], bf16)
            # q/k feature-major [d, t]: slot 0/1 = q heads {0,1}/{2,3}, 2/3 = k
            qk_t = [
                [persist.tile([128, 512], bf16, name=f"qk{s}_{tb}") for tb in range(4)]
                for s in range(4)
            ]
            # v token-major per 128-token tile; col D holds ones (denominator)
            v_t = [
                persist.tile([128, HL, D + 1], bf16, name=f"v{tt}") for tt in range(16)
            ]
            # head-pair stacked normalized y per 512-token block
            y2_t = [
                persist.tile([128, HL // 2, 512], bf16, name=f"y2{b_}")
                for b_ in range(4)
            ]
            warm = persist.tile([128, 512], bf16)

            # input DMAs: the first token block and the q/k weights for heads
            # 0/1 are the critical path, so their issues are spread across
            # four otherwise-idle engine queues to run in parallel; the rest
            # streams on the sync queue in first-use order
            nc.vector.memset(warm, 0.125)  # first: dummy matmuls wait on it
            nc.sync.dma_start(wqk_sb[:, :, 0:128], wqk[:, :, 0:128])
            nc.sync.dma_start(wqk_sb[:, :, 256:384], wqk[:, :, 256:384])
            for kt in range(4):
                nc.gpsimd.dma_start(x_sb[:, kt, 0:512], x_in[:, kt, 0:512])
            for kt in range(4, KT):
                nc.scalar.dma_start(x_sb[:, kt, 0:512], x_in[:, kt, 0:512])
            nc.sync.dma_start(wv_sb, wv[:, :, :])
            nc.sync.dma_start(wqk_sb[:, :, 128:256], wqk[:, :, 128:256])
            nc.sync.dma_start(wqk_sb[:, :, 384:512], wqk[:, :, 384:512])
            for tb in range(1, 4):
                nc.sync.dma_start(
                    x_sb[:, :, tb * 512 : (tb + 1) * 512],
                    x_in[:, :, tb * 512 : (tb + 1) * 512],
                )
            nc.sync.dma_start(wp_sb, wp[:, :, :])

            # constants
            ones_b = persist.tile([128, HL, 1], bf16)
            nc.vector.memset(ones_b, 1.0)
            for tt in range(16):
                nc.vector.tensor_copy(v_t[tt][:, :, D : D + 1], ones_b)
            # lower-triangular 0/1 mask for the diagonal 128x128 blocks
            tri32 = persist.tile([128, 128], f32)
            nc.vector.memset(tri32, 1.0)
            nc.gpsimd.affine_select(
                out=tri32,
                in_=tri32,
                pattern=[[1, 128]],
                channel_multiplier=-1,
                base=0,
                compare_op=mybir.AluOpType.is_ge,
                fill=0.0,
            )
            tri = persist.tile([128, 128], bf16)
            nc.vector.tensor_copy(tri, tri32)
            # ones row at partition D for the K=1 denominator broadcast
            onesbc32 = persist.tile([D + 1, D], f32)
            nc.vector.memset(onesbc32[D : D + 1, :], 1.0)
            onesbc = persist.tile([D + 1, D], f32r)
            nc.vector.tensor_copy(onesbc[D : D + 1, :], onesbc32[D : D + 1, :])

            with (
                tc.tile_pool(name="attp", bufs=14) as attp,
                tc.tile_pool(name="attsmall", bufs=2) as attsmall,
                tc.tile_pool(name="projp", bufs=2) as projp,
                tc.tile_pool(name="ps_st", bufs=2, space="PSUM") as ps_st,
                tc.tile_pool(name="ps_y", bufs=2, space="PSUM") as ps_y,
                tc.tile_pool(name="ps_share", bufs=2, space="PSUM") as ps_share,
            ):
                def warm_mms(n):
                    # dummy matmuls: keep the PE busy (HAM clock gate at full
                    # rate) across spots where it would otherwise idle
                    for i in range(n):
                        wps = ps_share.tile(
                            [128, 512], f32, tag="share", name="warm_ps"
                        )
                        nc.tensor.matmul(
                            wps, warm[:, 0:128], warm, start=True, stop=True
                        )

                warm_mms(6)

                def qk_chunk(tb, jt):
                    def go():
                        qk_ps = ps_share.tile(
                            [128, 512], f32, tag="share", name="qk_ps"
                        )
                        for kt in range(KT):
                            nc.tensor.matmul(
                                qk_ps,
                                wqk_sb[:, kt, jt * 128 : (jt + 1) * 128],
                                x_sb[:, kt, tb * 512 : (tb + 1) * 512],
                                start=(kt == 0),
                                stop=(kt == KT - 1),
                            )
                        nc.vector.tensor_copy(qk_t[jt][tb], qk_ps)

                    return go

                def v_chunk(tb, tt2):
                    def go():
                        tt = tb * 4 + tt2
                        v_ps = ps_share.tile(
                            [128, HL * D], f32, tag="share", name="v_ps"
                        )
                        for kt in range(KT):
                            nc.tensor.matmul(
                                v_ps,
                                x_sb[:, kt, tt * 128 : (tt + 1) * 128],
                                wv_sb[:, kt, :],
                                start=(kt == 0),
                                stop=(kt == KT - 1),
                            )
                        nc.vector.tensor_copy(
                            v_t[tt][:, :, 0:D],
                            v_ps.rearrange("p (h d) -> p h d", h=HL),
                        )

                    return go

                def proj_chunk(blk, tt, prs=(0, 1), dst=None, split_copy=False):
                    def go():
                        o_sb = projp.tile([128, C], bf16, name="o_sb")
                        off = (tt % 4) * 128
                        for cb in range(2):
                            o_ps = ps_share.tile(
                                [128, 512], f32, tag="share", name="o_ps"
                            )
                            for i, pr in enumerate(prs):
                                nc.tensor.matmul(
                                    o_ps,
                                    y2_t[blk][:, pr, off : off + 128],
                                    wp_sb[:, pr, cb * 512 : (cb + 1) * 512],
                                    start=(i == 0),
                                    stop=(i == len(prs) - 1),
                                )
                            if split_copy and cb == 1:
                                nc.scalar.copy(
                                    o_sb[:, cb * 512 : (cb + 1) * 512], o_ps
                                )
                            else:
                                nc.vector.tensor_copy(
                                    o_sb[:, cb * 512 : (cb + 1) * 512], o_ps
                                )
                        d = out[tt * 128 : (tt + 1) * 128, :] if dst is None else dst
                        nc.sync.dma_start(d, o_sb)

                    return go

                # q/k for heads 0/1 of token block 0 up front; the rest of
                # block 0's qkv runs as early fillers inside the jq0 loop
                for jt in (0, 2):
                    qk_chunk(0, jt)()

                def s_pair(jq, h, p, ests):
                    qslot = h // 2
                    kslot = 2 + h // 2
                    base = (h % 2) * D
                    st = ps_st.tile([128, 2, 512], f32, name="st")
                    est = attp.tile([128, 2, 512], bf16, tag="est", name="est")
                    diag = 2 * p + 1 >= 4 * jq
                    for s in range(2):
                        j = 2 * p + s
                        w = max(0, (j - 4 * jq) * 128)
                        nc.tensor.matmul(
                            st[:, s, w:],
                            qk_t[kslot][j // 4][
                                base : base + D,
                                (j % 4) * 128 : (j % 4 + 1) * 128,
                            ],
                            qk_t[qslot][jq][base : base + D, w:],
                            start=True,
                            stop=True,
                        )
                    if not diag:
                        nc.scalar.activation(est, st, EXP, scale=SCALE)
                    else:
                        for s in range(2):
                            j = 2 * p + s
                            w = max(0, (j - 4 * jq) * 128)
                            nc.scalar.activation(
                                est[:, s, w:], st[:, s, w:], EXP, scale=SCALE
                            )
                            nc.vector.tensor_mul(
                                est[:, s, w : w + 128], est[:, s, w : w + 128], tri
                            )
                    ests.append(est)

                class Pending:
                    def __init__(self, jq, h, ests):
                        self.jq, self.h, self.ests = jq, h, ests
                        self.njt = 4 * (jq + 1)
                        self.p = 0
                        self.y_ps = ps_y.tile([D + 1, 512], f32, name="y_ps")

                    def step(self):
                        if self.p >= len(self.ests):
                            return False
                        est = self.ests[self.p]
                        for s in range(2):
                            j = 2 * self.p + s
                            w = max(0, (j - 4 * self.jq) * 128)
                            nc.tensor.matmul(
                                self.y_ps[:, w:],
                                v_t[j][:, self.h, :],
                                est[:, s, w:],
                                start=(j == 0),
                                stop=(j == self.njt - 1),
                            )
                        self.p += 1
                        return True

                    def finish(self):
                        while self.step():
                            pass
                        jq, h, y_ps = self.jq, self.h, self.y_ps
                        pr = h // 2
                        # normalize: row D of y_ps holds the denominator
                        r_sb = attsmall.tile([D + 1, 512], f32r, tag="rr")
                        nc.vector.tensor_copy(r_sb[D : D + 1, :], y_ps[D : D + 1, :])
                        rb_ps = ps_share.tile(
                            [D, 512], f32, tag="share", name="rb_ps"
                        )
                        nc.tensor.matmul(
                            rb_ps,
                            onesbc[D : D + 1, :],
                            r_sb[D : D + 1, :],
                            start=True,
                            stop=True,
                        )
                        rb_sb = attsmall.tile([D, 512], f32, tag="rb")
                        nc.vector.reciprocal_approx_fast(rb_sb, rb_ps)
                        if h % 2 == 0:
                            nc.vector.tensor_mul(
                                y2_t[jq][0:D, pr, :], y_ps[0:D, :], rb_sb
                            )
                        else:
                            y_lo = attsmall.tile([D, 512], bf16, tag="ylo")
                            nc.vector.tensor_mul(y_lo, y_ps[0:D, :], rb_sb)
                            nc.gpsimd.dma_start(y2_t[jq][D:128, pr, :], y_lo)

                pend = [None]

                def drain_pend():
                    if pend[0] is not None:
                        pend[0].finish()
                        pend[0] = None

                # filler gating: "qkv" chunks are safe anywhere; "proj" reads
                # y2_t[jq-1] whose last slice is written by the epilogue of
                # (jq-1, h3), issued at the drain ending h0's pair loop, so it
                # may only pop from h >= 1; "proj3a" reads y2_t[3] pr0 written
                # by the epilogues of (jq3, h0/h1), so it may only pop in h3
                min_h = {"qkv": 0, "proj": 1, "proj3a": 3}
                fillers = deque()
                for jq in range(4):
                    fillers.clear()
                    if jq == 0:
                        for tt2 in range(4):
                            fillers.append(("qkv", v_chunk(0, tt2)))
                        for jt in (1, 3):
                            fillers.append(("qkv", qk_chunk(0, jt)))
                    if jq < 3:
                        for jt in range(4):
                            fillers.append(("qkv", qk_chunk(jq + 1, jt)))
                        for tt2 in range(4):
                            fillers.append(("qkv", v_chunk(jq + 1, tt2)))
                    if jq > 0:
                        for tt2 in range(4):
                            fillers.append(
                                ("proj", proj_chunk(jq - 1, (jq - 1) * 4 + tt2))
                            )
                    if jq == 3:
                        for tt2 in range(4):
                            fillers.append(
                                ("proj3a", proj_chunk(3, 12 + tt2, prs=(0,)))
                            )
                    npair = 2 * (jq + 1)
                    nslots = HL * npair
                    fcount = len(fillers)
                    slot = 0
                    fdone = 0
                    # jq3 ends on an even head so the very last epilogue takes
                    # the short path (direct vector mul, no y_lo DMA hop)
                    hseq = (0, 1, 3, 2) if jq == 3 else range(HL)
                    for pos, h in enumerate(hseq):
                        ests = []
                        for p in range(npair):
                            s_pair(jq, h, p, ests)
                            slot += 1
                            want = fcount * slot // nslots
                            while (
                                fdone < want
                                and fillers
                                and pos >= min_h[fillers[0][0]]
                            ):
                                fillers.popleft()[1]()
                                fdone += 1
                            if pend[0] is not None:
                                pend[0].step()
                        drain_pend()
                        pend[0] = Pending(jq, h, ests)
                    if jq < 3:
                        while fillers:
                            fillers.popleft()[1]()

                # tail: PV of the last head interleaved with leftover fillers,
                # then the epilogue and the pr1 half of block 3's c_proj
                while (pend[0] is not None and pend[0].p < len(pend[0].ests)) or fillers:
                    if pend[0] is not None:
                        pend[0].step()
                    if fillers:
                        fillers.popleft()[1]()
                warm_mms(4)  # hold the PE clock through the epilogue chain
                drain_pend()
                for tt2 in range(4):
                    proj_chunk(
                        3,
                        12 + tt2,
                        prs=(1,),
                        dst=out_b[tt2 * 128 : (tt2 + 1) * 128, :],
                        split_copy=True,
                    )()

    nc.compile()
    return nc


def _get_nc():
    if "nc" not in _CACHE:
        _CACHE["nc"] = _build()
    return _CACHE["nc"]


def make_in_maps(x, w_attn, w_proj):
    x = np.asarray(x, np.float32)
    w_attn = np.asarray(w_attn, np.float32)
    w_proj = np.asarray(w_proj, np.float32)
    bf16 = ml_dtypes.bfloat16
    in_maps = []
    for c in range(N_CORES):
        b, hg = c // 4, c % 4
        hs = hg * HL * D  # 256 * hg
        xt = np.ascontiguousarray(x[b].T)  # [C, T]
        x_t = xt.reshape(KT, 128, T).transpose(1, 0, 2)
        wq = w_attn[hs : hs + HL * D, :]
        wk = w_attn[C + hs : C + hs + HL * D, :]
        wqkt = np.concatenate([wq, wk], 0).T  # [C, 512]
        wqk_t = wqkt.reshape(KT, 128, 2 * HL * D).transpose(1, 0, 2)
        wvt = w_attn[2 * C + hs : 2 * C + hs + HL * D, :].T  # [C, 256]
        wv_t = wvt.reshape(KT, 128, HL * D).transpose(1, 0, 2)
        # head-pair stacked rows: [128, HL//2, C]; partition p of pair pr is
        # local feature pr*128 + p (head 2*pr dims then head 2*pr+1 dims)
        wp_t = (
            w_proj[:, hs : hs + HL * D].T.reshape(HL // 2, 128, C).transpose(1, 0, 2)
        )
        in_maps.append(
            {
                "x_in": np.ascontiguousarray(x_t).astype(bf16),
                "wqk": np.ascontiguousarray(wqk_t).astype(bf16),
                "wv": np.ascontiguousarray(wv_t).astype(bf16),
                "wp": np.ascontiguousarray(wp_t).astype(bf16),
            }
        )
    return in_maps


def run(in_maps, **kwargs):
    nc = _get_nc()
    return run_bass_kernel_spmd(nc, in_maps, core_ids=list(range(N_CORES)), **kwargs)


def combine(results):
    out = np.zeros((B, T, C), np.float64)
    for c in range(N_CORES):
        out[c // 4] += results[c]["out"].astype(np.float64)
        # token block 3 was written pr-split: "out" rows 1536: hold the pr0
        # half, "out_b" the pr1 half
        out[c // 4][3 * 512 :] += results[c]["out_b"].astype(np.float64)
    return out.astype(np.float32)


def kernel(x, w_attn, w_proj):
    res = run(make_in_maps(x, w_attn, w_proj))
    return combine(res.results)


# revision 18
# speedup vs baseline: 1.3941x; 1.0199x over previous
"""Causal self-attention on 8 Trainium2 NeuronCores.

Sharding (batch + head-parallel): core c handles batch b = c // 4 and the
4 heads [hg*4, hg*4+4) where hg = c % 4.  Each core computes q/k/v from
column-sliced c_attn weights, full causal attention for its heads, and a
partial c_proj output from the matching row slice of w_proj; the host
sums the 4 partials per batch.

All matmul inputs are bf16 (fp32 PSUM accumulate).  The schedule keeps
the PE array continuously busy so the HAM clock gate stays at full rate:
 - inputs stream in bf16 with the first token block prioritized,
 - dummy warm-up matmuls run while the first DMAs land,
 - in the attention phase, S matmuls of head h interleave with PV
   matmuls of head h-1, with next-block QKV and previous-block c_proj
   matmuls sprinkled in as fillers, so exp latency (scalar engine)
   never stalls the PE.
"""

import sys

if "/opt/trn_rl_repo" not in sys.path:
    sys.path.insert(0, "/opt/trn_rl_repo")

from collections import deque

import ml_dtypes
import numpy as np

import concourse.mybir as mybir
from concourse import bacc
from concourse.bass_utils import run_bass_kernel_spmd
from concourse.tile import TileContext

B, T, C = 2, 2048, 1024
H, D = 16, 64
HL = 4  # heads per core
N_CORES = 8
KT = C // 128  # contraction tiles over the embedding dim
SCALE = 1.0 / 8.0  # 1/sqrt(D)

_CACHE = {}


def _build():
    f32 = mybir.dt.float32
    f32r = mybir.dt.float32r
    bf16 = mybir.dt.bfloat16
    EXP = mybir.ActivationFunctionType.Exp
    nc = bacc.Bacc("TRN2", target_bir_lowering=False, debug=False, num_devices=N_CORES)

    x_in = nc.dram_tensor("x_in", [128, KT, T], bf16, kind="ExternalInput")
    wqk = nc.dram_tensor("wqk", [128, KT, 2 * HL * D], bf16, kind="ExternalInput")
    wv = nc.dram_tensor("wv", [128, KT, HL * D], bf16, kind="ExternalInput")
    wp = nc.dram_tensor("wp", [128, HL // 2, C], bf16, kind="ExternalInput")
    out = nc.dram_tensor("out", [T, C], bf16, kind="ExternalOutput")
    # pr1 half of token block 3 lands separately so its c_proj matmuls can
    # start before the last head's epilogue; the host sums the two halves
    out_b = nc.dram_tensor("out_b", [512, C], bf16, kind="ExternalOutput")

    with TileContext(nc) as tc:
        with tc.tile_pool(name="persist", bufs=1) as persist:
            x_sb = persist.tile([128, KT, T], bf16)
            wqk_sb = persist.tile([128, KT, 2 * HL * D], bf16)
            wv_sb = persist.tile([128, KT, HL * D], bf16)
            wp_sb = persist.tile([128, HL // 2, C], bf16)
            # q/k feature-major [d, t]: slot 0/1 = q heads {0,1}/{2,3}, 2/3 = k
            qk_t = [
                [persist.tile([128, 512], bf16, name=f"qk{s}_{tb}") for tb in range(4)]
                for s in range(4)
            ]
            # v token-major per 128-token tile; col D holds ones (denominator)
            v_t = [
                persist.tile([128, HL, D + 1], bf16, name=f"v{tt}") for tt in range(16)
            ]
            # head-pair stacked normalized y per 512-token block
            y2_t = [
                persist.tile([128, HL // 2, 512], bf16, name=f"y2{b_}")
                for b_ in range(4)
            ]
            warm = persist.tile([128, 512], bf16)

            # input DMAs: the first token block and the q/k weights for heads
            # 0/1 are the critical path, so their issues are spread across
            # four otherwise-idle engine queues to run in parallel; the rest
            # streams on the sync queue in first-use order
            nc.vector.memset(warm, 0.125)  # first: dummy matmuls wait on it
            nc.sync.dma_start(wqk_sb[:, :, 0:128], wqk[:, :, 0:128])
            nc.sync.dma_start(wqk_sb[:, :, 256:384], wqk[:, :, 256:384])
            for kt in range(4):
                nc.gpsimd.dma_start(x_sb[:, kt, 0:512], x_in[:, kt, 0:512])
            for kt in range(4, KT):
                nc.scalar.dma_start(x_sb[:, kt, 0:512], x_in[:, kt, 0:512])
            nc.sync.dma_start(wv_sb, wv[:, :, :])
            nc.sync.dma_start(wqk_sb[:, :, 128:256], wqk[:, :, 128:256])
            nc.sync.dma_start(wqk_sb[:, :, 384:512], wqk[:, :, 384:512])
            for tb in range(1, 4):
                nc.sync.dma_start(
                    x_sb[:, :, tb * 512 : (tb + 1) * 512],
                    x_in[:, :, tb * 512 : (tb + 1) * 512],
                )
            nc.sync.dma_start(wp_sb, wp[:, :, :])

            # constants
            ones_b = persist.tile([128, HL, 1], bf16)
            nc.vector.memset(ones_b, 1.0)
            for tt in range(16):
                nc.vector.tensor_copy(v_t[tt][:, :, D : D + 1], ones_b)
            # lower-triangular 0/1 mask for the diagonal 128x128 blocks
            tri32 = persist.tile([128, 128], f32)
            nc.vector.memset(tri32, 1.0)
            nc.gpsimd.affine_select(
                out=tri32,
                in_=tri32,
                pattern=[[1, 128]],
                channel_multiplier=-1,
                base=0,
                compare_op=mybir.AluOpType.is_ge,
                fill=0.0,
            )
            tri = persist.tile([128, 128], bf16)
            nc.vector.tensor_copy(tri, tri32)
            # ones row at partition D for the K=1 denominator broadcast
            onesbc32 = persist.tile([D + 1, D], f32)
            nc.vector.memset(onesbc32[D : D + 1, :], 1.0)
            onesbc = persist.tile([D + 1, D], f32r)
            nc.vector.tensor_copy(onesbc[D : D + 1, :], onesbc32[D : D + 1, :])

            with (
                tc.tile_pool(name="attp", bufs=14) as attp,
                tc.tile_pool(name="attsmall", bufs=2) as attsmall,
                tc.tile_pool(name="projp", bufs=2) as projp,
                tc.tile_pool(name="ps_st", bufs=2, space="PSUM") as ps_st,
                tc.tile_pool(name="ps_y", bufs=2, space="PSUM") as ps_y,
                tc.tile_pool(name="ps_share", bufs=2, space="PSUM") as ps_share,
            ):
                def warm_mms(n):
                    # dummy matmuls: keep the PE busy (HAM clock gate at full
                    # rate) across spots where it would otherwise idle
                    for i in range(n):
                        wps = ps_share.tile(
                            [128, 512], f32, tag="share", name="warm_ps"
                        )
                        nc.tensor.matmul(
                            wps, warm[:, 0:128], warm, start=True, stop=True
                        )

                warm_mms(12)

                def qk_chunk(tb, jt):
                    def go():
                        qk_ps = ps_share.tile(
                            [128, 512], f32, tag="share", name="qk_ps"
                        )
                        for kt in range(KT):
                            nc.tensor.matmul(
                                qk_ps,
                                wqk_sb[:, kt, jt * 128 : (jt + 1) * 128],
                                x_sb[:, kt, tb * 512 : (tb + 1) * 512],
                                start=(kt == 0),
                                stop=(kt == KT - 1),
                            )
                        nc.vector.tensor_copy(qk_t[jt][tb], qk_ps)

                    return go

                def v_chunk(tb, tt2):
                    def go():
                        tt = tb * 4 + tt2
                        v_ps = ps_share.tile(
                            [128, HL * D], f32, tag="share", name="v_ps"
                        )
                        for kt in range(KT):
                            nc.tensor.matmul(
                                v_ps,
                                x_sb[:, kt, tt * 128 : (tt + 1) * 128],
                                wv_sb[:, kt, :],
                                start=(kt == 0),
                                stop=(kt == KT - 1),
                            )
                        nc.vector.tensor_copy(
                            v_t[tt][:, :, 0:D],
                            v_ps.rearrange("p (h d) -> p h d", h=HL),
                        )

                    return go

                def proj_chunk(blk, tt, prs=(0, 1), dst=None, split_copy=False):
                    def go():
                        o_sb = projp.tile([128, C], bf16, name="o_sb")
                        off = (tt % 4) * 128
                        for cb in range(2):
                            o_ps = ps_share.tile(
                                [128, 512], f32, tag="share", name="o_ps"
                            )
                            for i, pr in enumerate(prs):
                                nc.tensor.matmul(
                                    o_ps,
                                    y2_t[blk][:, pr, off : off + 128],
                                    wp_sb[:, pr, cb * 512 : (cb + 1) * 512],
                                    start=(i == 0),
                                    stop=(i == len(prs) - 1),
                                )
                            if split_copy and cb == 1:
                                nc.scalar.copy(
                                    o_sb[:, cb * 512 : (cb + 1) * 512], o_ps
                                )
                            else:
                                nc.vector.tensor_copy(
                                    o_sb[:, cb * 512 : (cb + 1) * 512], o_ps
                                )
                        d = out[tt * 128 : (tt + 1) * 128, :] if dst is None else dst
                        nc.sync.dma_start(d, o_sb)

                    return go

                # q/k for heads 0/1 of token block 0 up front; the rest of
                # block 0's qkv runs as early fillers inside the jq0 loop
                for jt in (0, 2):
                    qk_chunk(0, jt)()

                def s_pair(jq, h, p, ests):
                    qslot = h // 2
                    kslot = 2 + h // 2
                    base = (h % 2) * D
                    st = ps_st.tile([128, 2, 512], f32, name="st")
                    est = attp.tile([128, 2, 512], bf16, tag="est", name="est")
                    diag = 2 * p + 1 >= 4 * jq
                    for s in range(2):
                        j = 2 * p + s
                        w = max(0, (j - 4 * jq) * 128)
                        nc.tensor.matmul(
                            st[:, s, w:],
                            qk_t[kslot][j // 4][
                                base : base + D,
                                (j % 4) * 128 : (j % 4 + 1) * 128,
                            ],
                            qk_t[qslot][jq][base : base + D, w:],
                            start=True,
                            stop=True,
                        )
                    if not diag:
                        nc.scalar.activation(est, st, EXP, scale=SCALE)
                    else:
                        for s in range(2):
                            j = 2 * p + s
                            w = max(0, (j - 4 * jq) * 128)
                            nc.scalar.activation(
                                est[:, s, w:], st[:, s, w:], EXP, scale=SCALE
                            )
                            nc.vector.tensor_mul(
                                est[:, s, w : w + 128], est[:, s, w : w + 128], tri
                            )
                    ests.append(est)

                class Pending:
                    def __init__(self, jq, h, ests):
                        self.jq, self.h, self.ests = jq, h, ests
                        self.njt = 4 * (jq + 1)
                        self.p = 0
                        self.y_ps = ps_y.tile([D + 1, 512], f32, name="y_ps")

                    def step(self):
                        if self.p >= len(self.ests):
                            return False
                        est = self.ests[self.p]
                        for s in range(2):
                            j = 2 * self.p + s
                            w = max(0, (j - 4 * self.jq) * 128)
                            nc.tensor.matmul(
                                self.y_ps[:, w:],
                                v_t[j][:, self.h, :],
                                est[:, s, w:],
                                start=(j == 0),
                                stop=(j == self.njt - 1),
                            )
                        self.p += 1
                        return True

                    def finish(self):
                        while self.step():
                            pass
                        jq, h, y_ps = self.jq, self.h, self.y_ps
                        pr = h // 2
                        # normalize: row D of y_ps holds the denominator
                        r_sb = attsmall.tile([D + 1, 512], f32r, tag="rr")
                        nc.vector.tensor_copy(r_sb[D : D + 1, :], y_ps[D : D + 1, :])
                        rb_ps = ps_share.tile(
                            [D, 512], f32, tag="share", name="rb_ps"
                        )
                        nc.tensor.matmul(
                            rb_ps,
                            onesbc[D : D + 1, :],
                            r_sb[D : D + 1, :],
                            start=True,
                            stop=True,
                        )
                        rb_sb = attsmall.tile([D, 512], f32, tag="rb")
                        nc.vector.reciprocal_approx_fast(rb_sb, rb_ps)
                        if h % 2 == 0:
                            nc.vector.tensor_mul(
                                y2_t[jq][0:D, pr, :], y_ps[0:D, :], rb_sb
                            )
                        else:
                            y_lo = attsmall.tile([D, 512], bf16, tag="ylo")
                            nc.vector.tensor_mul(y_lo, y_ps[0:D, :], rb_sb)
                            nc.gpsimd.dma_start(y2_t[jq][D:128, pr, :], y_lo)

                pend = [None]

                def drain_pend():
                    if pend[0] is not None:
                        pend[0].finish()
                        pend[0] = None

                # filler gating: "qkv" chunks are safe anywhere; "proj" reads
                # y2_t[jq-1] whose last slice is written by the epilogue of
                # (jq-1, h3), issued at the drain ending h0's pair loop, so it
                # may only pop from h >= 1; "proj3a" reads y2_t[3] pr0 written
                # by the epilogues of (jq3, h0/h1), so it may only pop in h3
                min_h = {"qkv": 0, "proj": 1, "proj3a": 3}
                fillers = deque()
                for jq in range(4):
                    fillers.clear()
                    if jq == 0:
                        for tt2 in range(4):
                            fillers.append(("qkv", v_chunk(0, tt2)))
                        for jt in (1, 3):
                            fillers.append(("qkv", qk_chunk(0, jt)))
                    if jq < 3:
                        for jt in range(4):
                            fillers.append(("qkv", qk_chunk(jq + 1, jt)))
                        for tt2 in range(4):
                            fillers.append(("qkv", v_chunk(jq + 1, tt2)))
                    if jq > 0:
                        for tt2 in range(4):
                            fillers.append(
                                ("proj", proj_chunk(jq - 1, (jq - 1) * 4 + tt2))
                            )
                    if jq == 3:
                        for tt2 in range(4):
                            fillers.append(
                                ("proj3a", proj_chunk(3, 12 + tt2, prs=(0,)))
                            )
                    npair = 2 * (jq + 1)
                    nslots = HL * npair
                    fcount = len(fillers)
                    slot = 0
                    fdone = 0
                    # jq3 ends on an even head so the very last epilogue takes
                    # the short path (direct vector mul, no y_lo DMA hop)
                    hseq = (0, 1, 3, 2) if jq == 3 else range(HL)
                    for pos, h in enumerate(hseq):
                        ests = []
                        for p in range(npair):
                            s_pair(jq, h, p, ests)
                            slot += 1
                            want = fcount * slot // nslots
                            while (
                                fdone < want
                                and fillers
                                and pos >= min_h[fillers[0][0]]
                            ):
                                fillers.popleft()[1]()
                                fdone += 1
                            if pend[0] is not None:
                                pend[0].step()
                        drain_pend()
                        pend[0] = Pending(jq, h, ests)
                    if jq < 3:
                        while fillers:
                            fillers.popleft()[1]()

                # tail: PV of the last head interleaved with leftover fillers,
                # then the epilogue and the pr1 half of block 3's c_proj
                while (pend[0] is not None and pend[0].p < len(pend[0].ests)) or fillers:
                    if pend[0] is not None:
                        pend[0].step()
                    if fillers:
                        fillers.popleft()[1]()
                warm_mms(4)  # hold the PE clock through the epilogue chain
                drain_pend()
                for tt2 in range(4):
                    proj_chunk(
                        3,
                        12 + tt2,
                        prs=(1,),
                        dst=out_b[tt2 * 128 : (tt2 + 1) * 128, :],
                        split_copy=True,
                    )()

    nc.compile()
    return nc


def _get_nc():
    if "nc" not in _CACHE:
        _CACHE["nc"] = _build()
    return _CACHE["nc"]


def make_in_maps(x, w_attn, w_proj):
    x = np.asarray(x, np.float32)
    w_attn = np.asarray(w_attn, np.float32)
    w_proj = np.asarray(w_proj, np.float32)
    bf16 = ml_dtypes.bfloat16
    in_maps = []
    for c in range(N_CORES):
        b, hg = c // 4, c % 4
        hs = hg * HL * D  # 256 * hg
        xt = np.ascontiguousarray(x[b].T)  # [C, T]
        x_t = xt.reshape(KT, 128, T).transpose(1, 0, 2)
        wq = w_attn[hs : hs + HL * D, :]
        wk = w_attn[C + hs : C + hs + HL * D, :]
        wqkt = np.concatenate([wq, wk], 0).T  # [C, 512]
        wqk_t = wqkt.reshape(KT, 128, 2 * HL * D).transpose(1, 0, 2)
        wvt = w_attn[2 * C + hs : 2 * C + hs + HL * D, :].T  # [C, 256]
        wv_t = wvt.reshape(KT, 128, HL * D).transpose(1, 0, 2)
        # head-pair stacked rows: [128, HL//2, C]; partition p of pair pr is
        # local feature pr*128 + p (head 2*pr dims then head 2*pr+1 dims)
        wp_t = (
            w_proj[:, hs : hs + HL * D].T.reshape(HL // 2, 128, C).transpose(1, 0, 2)
        )
        in_maps.append(
            {
                "x_in": np.ascontiguousarray(x_t).astype(bf16),
                "wqk": np.ascontiguousarray(wqk_t).astype(bf16),
                "wv": np.ascontiguousarray(wv_t).astype(bf16),
                "wp": np.ascontiguousarray(wp_t).astype(bf16),
            }
        )
    return in_maps


def run(in_maps, **kwargs):
    nc = _get_nc()
    return run_bass_kernel_spmd(nc, in_maps, core_ids=list(range(N_CORES)), **kwargs)


def combine(results):
    out = np.zeros((B, T, C), np.float64)
    for c in range(N_CORES):
        out[c // 4] += results[c]["out"].astype(np.float64)
        # token block 3 was written pr-split: "out" rows 1536: hold the pr0
        # half, "out_b" the pr1 half
        out[c // 4][3 * 512 :] += results[c]["out_b"].astype(np.float64)
    return out.astype(np.float32)


def kernel(x, w_attn, w_proj):
    res = run(make_in_maps(x, w_attn, w_proj))
    return combine(res.results)
